# revision 1
# baseline (speedup 1.0000x reference)
"""Multi-head attention (16 heads, d=64, d_model=1024, SL=2048, BS=2) on 8
Trainium2 NeuronCores.

Sharding: core c handles batch b = c // 4 and heads [4*(c%4), 4*(c%4)+4).
Each core computes a partial output y_c[2048, 1024] (its 4 heads' contribution
through Wo for its batch); the host sums the 4 partials per batch.

Host-side prep feeds activations TRANSPOSED ([d_model, seq]) so every on-chip
matmul has its contraction dim on partitions; no on-chip transposes needed.

On-chip dataflow per core (all matmuls fp32r: full PE rate, ~1e-4 rel err):
  Q^T[256,2048], K^T[256,2048]  (psum-accumulated over 8 d_model chunks)
  V[2048,256] interleaved with ones columns ([V_h | 1] per head, 65 cols)
  per (q-chunk 512, head): S^T[k,q] tiles -> exp (ACT, no max-subtraction:
  |scores| < ~25 so fp32 exp is exact-enough) -> attnU^T[65,512] accumulated
  over 16 k-tiles; row 64 = softmax denominator l.
  normalize: recip(l) -> PE broadcast to 64 rows -> DVE multiply -> A^T
  O-proj: y[q,1024] += A^T-chunk.T @ Wo^T-chunk.
"""

import os
import sys
for _p in ("/opt/trn_rl_repo", "/root/.axon_site/_ro/trn_rl_repo"):
    if os.path.isdir(_p) and _p not in sys.path:
        sys.path.insert(0, _p)

import numpy as np

import concourse.bass as bass
import concourse.tile as tile
from concourse import bacc, mybir
from concourse.bass_utils import run_bass_kernel_spmd

N_CORES = 8
SL = 2048
BS = 2
DM = 1024          # d_model
H = 16             # total heads
DH = 64            # head dim
HPC = 4            # heads per core
IC = HPC * DH      # per-core inner dim = 256
F32 = mybir.dt.float32
BF16 = mybir.dt.bfloat16
F32R = mybir.dt.float32r
Exp = mybir.ActivationFunctionType.Exp

N_DMC = DM // 128          # 8 d_model chunks
N_KT = SL // 128           # 16 k tiles
N_QC = SL // 512           # 4 q chunks
N_QT = SL // 128           # 16 q tiles (o-proj)
VW = 65                    # V columns per head incl. ones column
VBLK = HPC * VW            # 260 V columns per k-tile block


def build_kernel(taps=False, reps=1):
    nc = bacc.Bacc("TRN2", target_bir_lowering=False, debug=False,
                   num_devices=N_CORES)
    qT = nc.dram_tensor("qT", [DM, SL], BF16, kind="ExternalInput").ap()
    kT = nc.dram_tensor("kT", [DM, SL], BF16, kind="ExternalInput").ap()
    vT = nc.dram_tensor("vT", [DM, SL], BF16, kind="ExternalInput").ap()
    wqT = nc.dram_tensor("wqT", [DM, IC], BF16, kind="ExternalInput").ap()
    wkT = nc.dram_tensor("wkT", [DM, IC], BF16, kind="ExternalInput").ap()
    wvT = nc.dram_tensor("wvT", [DM, IC], BF16, kind="ExternalInput").ap()
    woT = nc.dram_tensor("woT", [IC, DM], F32R, kind="ExternalInput").ap()
    Y = nc.dram_tensor("Y", [SL, DM], F32, kind="ExternalOutput").ap()
    tap_aps = None
    if taps:
        tap_aps = {
            nm: nc.dram_tensor(nm, shp, F32, kind="ExternalOutput").ap()
            for nm, shp in (("DQT", [128, SL]), ("DKT", [128, SL]),
                            ("DV", [128, N_KT * VBLK]), ("DAT", [128, SL]),
                            ("DPT", [128, 512]), ("DAU", [VW, 512]),
                            ("DRC", [1, 512]), ("DRB", [64, 512]))
        }

    with tile.TileContext(nc) as tc:
        _build_body(nc, tc, qT, kT, vT, wqT, wkT, wvT, woT, Y, tap_aps, reps)
    nc.compile()
    return nc


def _build_body(nc, tc, qT, kT, vT, wqT, wkT, wvT, woT, Y, tap_aps=None, reps=1):
    import contextlib
    ctx = contextlib.ExitStack()
    with ctx:
        wpool = ctx.enter_context(tc.tile_pool(name="w", bufs=1))
        xin = ctx.enter_context(tc.tile_pool(name="xin", bufs=8))
        qk = ctx.enter_context(tc.tile_pool(name="qk", bufs=1))
        vpool = ctx.enter_context(tc.tile_pool(name="v", bufs=1))
        ptp = ctx.enter_context(tc.tile_pool(name="pt", bufs=4))
        atp = ctx.enter_context(tc.tile_pool(name="at", bufs=1))
        ypool = ctx.enter_context(tc.tile_pool(name="y", bufs=2))
        misc = ctx.enter_context(tc.tile_pool(name="misc", bufs=2))
        ps = ctx.enter_context(tc.tile_pool(name="ps", bufs=2, space="PSUM"))
        psu = ctx.enter_context(tc.tile_pool(name="psu", bufs=3, space="PSUM"))
        psy = ctx.enter_context(tc.tile_pool(name="psy", bufs=1, space="PSUM"))

        # ---- weights (tiles allocated up-front; DMAs issued near first use)
        w_sb = {}
        w_dram = {"wq": wqT, "wk": wkT, "wv": wvT}
        for name in ("wq", "wk", "wv"):
            w_sb[name] = wpool.tile([128, N_DMC * IC], BF16, tag=name,
                                    name=name)

        def load_w(name):
            wT = w_dram[name]
            t = w_sb[name]
            nc.scalar.dma_start(
                out=t[:].rearrange("p (c f) -> p c f", c=N_DMC),
                in_=wT.rearrange("(c p) f -> p c f", p=128))

        wo_sb = []
        for i in range(2):
            t = wpool.tile([128, DM], F32R, tag=f"wo{i}", name=f"wo{i}")
            wo_sb.append(t)

        def load_wo():
            for i in range(2):
                nc.scalar.dma_start(out=wo_sb[i][:],
                                  in_=woT[i * 128:(i + 1) * 128, :])

        ones_f32 = misc.tile([128, DH], F32, tag="ones_f32")
        nc.vector.memset(ones_f32[:], 1.0)
        ones_sb = misc.tile([1, DH], F32R, tag="ones")
        nc.vector.tensor_copy(ones_sb[:], ones_f32[0:1, :])

        # ---- long-lived activations ----
        QT = [qk.tile([128, SL], F32R, tag=f"qt{p}", name=f"qt{p}") for p in range(2)]
        KT = [qk.tile([128, SL], F32R, tag=f"kt{p}", name=f"kt{p}") for p in range(2)]
        AT = [atp.tile([128, SL], F32R, tag=f"at{p}", name=f"at{p}") for p in range(2)]
        V = vpool.tile([128, N_KT * VBLK], F32R, tag="vsb")
        # ones columns of V (col 64 of each head's 65-wide block)
        for h in range(HPC):
            nc.vector.tensor_copy(V[:, h * VW + 64::VBLK],
                                  ones_f32[:, 0:N_KT])

        # ---- projections ----
        def proj_qk(xdram, wname, out_tiles):
            chunks = []
            for c in range(N_DMC):
                xt = xin.tile([128, SL], BF16, tag="xin")
                nc.sync.dma_start(out=xt[:], in_=xdram[c * 128:(c + 1) * 128, :])
                chunks.append(xt)
            for hp in range(2):
                for tcq in range(N_QC):    # 512-token chunks
                    acc = psu.tile([128, 512], F32, tag="accu")
                    for c in range(N_DMC):
                        nc.tensor.matmul(
                            acc[:],
                            w_sb[wname][:, c * IC + hp * 128:
                                        c * IC + (hp + 1) * 128],
                            chunks[c][:, tcq * 512:(tcq + 1) * 512],
                            start=(c == 0), stop=(c == N_DMC - 1))
                    nc.vector.tensor_copy(
                        out_tiles[hp][:, tcq * 512:(tcq + 1) * 512],
                        acc[:])

        def proj_v():
            chunks = []
            for c in range(N_DMC):
                xt = xin.tile([128, SL], BF16, tag="xin")
                nc.scalar.dma_start(out=xt[:], in_=vT[c * 128:(c + 1) * 128, :])
                chunks.append(xt)
            for kt in range(N_KT):
                acc = psu.tile([128, 512], F32, tag="accu")
                for c in range(N_DMC):
                    nc.tensor.matmul(
                        acc[:, 0:IC],
                        chunks[c][:, kt * 128:(kt + 1) * 128],
                        w_sb["wv"][:, c * IC:(c + 1) * IC],
                        start=(c == 0), stop=(c == N_DMC - 1))
                for h in range(HPC):
                    nc.vector.tensor_copy(
                        V[:, kt * VBLK + h * VW:kt * VBLK + h * VW + 64],
                        acc[:, h * 64:(h + 1) * 64])

        for _rep in range(reps):
            load_w("wk")
            proj_qk(kT, "wk", KT)
            load_w("wq")
            proj_qk(qT, "wq", QT)
            load_w("wv")
            proj_v()
            load_wo()

            # ---- attention + o-proj, per q-chunk ----
            for qc in range(N_QC):
                for pair in range(2):
                    au = []
                    for hl in range(2):        # head-local within pair
                        au.append(psu.tile([VW, 512], F32, tag="accu", name=f"au{hl}"))
                    for g in range(N_KT // 2):
                        pts = []
                        for hl in range(2):
                            s = ps.tile([128, 1024], F32, tag="sgrp")
                            for j in range(2):
                                kt = 2 * g + j
                                nc.tensor.matmul(
                                    s[:, j * 512:(j + 1) * 512],
                                    KT[pair][hl * 64:(hl + 1) * 64,
                                             kt * 128:(kt + 1) * 128],
                                    QT[pair][hl * 64:(hl + 1) * 64,
                                             qc * 512:(qc + 1) * 512],
                                    start=True, stop=True)
                            p = ptp.tile([128, 1024], F32R, tag="pt")
                            nc.scalar.activation(p[:], s[:], Exp)
                            if tap_aps and qc == 0 and pair == 0 and g == 0 and hl == 0:
                                nc.sync.dma_start(out=tap_aps["DPT"][:],
                                                  in_=p[:, 0:512].bitcast(F32))
                            pts.append(p)
                        for hl in range(2):
                            h = pair * 2 + hl
                            for j in range(2):
                                kt = 2 * g + j
                                nc.tensor.matmul(
                                    au[hl][:],
                                    V[:, kt * VBLK + h * VW:kt * VBLK + (h + 1) * VW],
                                    pts[hl][:, j * 512:(j + 1) * 512],
                                    start=(kt == 0), stop=(kt == N_KT - 1))
                    if tap_aps and qc == 0 and pair == 0:
                        dau_sb = misc.tile([VW, 512], F32, tag="dau")
                        nc.vector.tensor_copy(dau_sb[:], au[0][:])
                        nc.sync.dma_start(out=tap_aps["DAU"][:], in_=dau_sb[:])
                    # normalize: A^T = attnU[0:64] * (1/l) broadcast
                    for hl in range(2):
                        l_sb = misc.tile([1, 512], F32, tag="l_sb")
                        nc.vector.tensor_copy(l_sb[:], au[hl][64:65, :])
                        rc = misc.tile([1, 512], F32, tag="rc")
                        nc.vector.reciprocal_approx_fast(out=rc[:], in_=l_sb[:])
                        rb = misc.tile([64, 512], F32, tag="rb")
                        nc.gpsimd.partition_broadcast(rb[:], rc[:])
                        if tap_aps and qc == 0 and pair == 0 and hl == 0:
                            nc.sync.dma_start(out=tap_aps["DRC"][:], in_=rc[:])
                            nc.sync.dma_start(out=tap_aps["DRB"][:], in_=rb[:])
                        nc.vector.tensor_mul(
                            AT[pair][hl * 64:(hl + 1) * 64,
                                     qc * 512:(qc + 1) * 512],
                            au[hl][0:64, :], rb[:])
                if tap_aps and qc == N_QC - 1:
                    nc.sync.dma_start(out=tap_aps["DQT"][:], in_=QT[0][:].bitcast(F32))
                    nc.sync.dma_start(out=tap_aps["DKT"][:], in_=KT[0][:].bitcast(F32))
                    nc.sync.dma_start(out=tap_aps["DV"][:], in_=V[:].bitcast(F32))
                    nc.sync.dma_start(out=tap_aps["DAT"][:], in_=AT[0][:].bitcast(F32))
                # o-proj for this q-chunk (all 4 heads ready)
                for qt in range(4 * qc, 4 * (qc + 1)):
                    y_sb = ypool.tile([128, DM], F32, tag="ysb")
                    for mh in range(2):        # output halves of 1024
                        yp = psy.tile([128, 512], F32, tag="yp")
                        for ich in range(2):   # i chunks (AT0, AT1)
                            nc.tensor.matmul(
                                yp[:],
                                AT[ich][:, qt * 128:(qt + 1) * 128],
                                wo_sb[ich][:, mh * 512:(mh + 1) * 512],
                                start=(ich == 0), stop=(ich == 1))
                        nc.vector.tensor_copy(y_sb[:, mh * 512:(mh + 1) * 512],
                                              yp[:])
                    nc.gpsimd.dma_start(out=Y[qt * 128:(qt + 1) * 128, :],
                                        in_=y_sb[:])


_NC_CACHE = None


def _get_nc():
    global _NC_CACHE
    if _NC_CACHE is None:
        _NC_CACHE = build_kernel()
    return _NC_CACHE


def make_in_maps(query, keys, values, Wq, Wk, Wv, Wo):
    query = np.ascontiguousarray(query, dtype=np.float32)
    keys = np.ascontiguousarray(keys, dtype=np.float32)
    values = np.ascontiguousarray(values, dtype=np.float32)
    import ml_dtypes
    bf16 = ml_dtypes.bfloat16
    xTs = {}
    for b in range(BS):
        xTs[b] = (
            np.ascontiguousarray(query[:, b, :].T.astype(bf16)),
            np.ascontiguousarray(keys[:, b, :].T.astype(bf16)),
            np.ascontiguousarray(values[:, b, :].T.astype(bf16)),
        )
    wTs = {}
    bf16 = __import__("ml_dtypes").bfloat16
    for g in range(N_CORES // BS):
        sl = slice(g * IC, (g + 1) * IC)
        wTs[g] = (
            np.ascontiguousarray(np.asarray(Wq, dtype=np.float32)[sl, :].T.astype(bf16)),
            np.ascontiguousarray(np.asarray(Wk, dtype=np.float32)[sl, :].T.astype(bf16)),
            np.ascontiguousarray(np.asarray(Wv, dtype=np.float32)[sl, :].T.astype(bf16)),
            np.ascontiguousarray(np.asarray(Wo, dtype=np.float32)[:, sl].T),
        )
    in_maps = []
    for c in range(N_CORES):
        b, g = c // 4, c % 4
        qTb, kTb, vTb = xTs[b]
        wq, wk, wv, wo = wTs[g]
        in_maps.append({"qT": qTb, "kT": kTb, "vT": vTb,
                        "wqT": wq, "wkT": wk, "wvT": wv, "woT": wo})
    return in_maps


def assemble_output(results):
    out = np.zeros((SL, BS, DM), dtype=np.float32)
    for c in range(N_CORES):
        b = c // 4
        out[:, b, :] += results[c]["Y"]
    return out


def kernel(query, keys, values, Wq, Wk, Wv, Wo):
    nc = _get_nc()
    in_maps = make_in_maps(query, keys, values, Wq, Wk, Wv, Wo)
    res = run_bass_kernel_spmd(nc, in_maps, list(range(N_CORES)))
    return assemble_output(res.results)

